# revision 1
# baseline (speedup 1.0000x reference)
"""Cross-attention kernel for Trainium2, sharded over 8 NeuronCores.

Problem (hardcoded shapes): B=2, N=4096, M=1024, DIM=1024, H=16, D=64.
  q = rms_norm(x @ Wq.T + bq)        per-head, gamma gq, eps 1e-6
  k = rms_norm(ctx @ Wk.T + bk)      (Wk = first half of Wkv)
  v = ctx @ Wv.T + bv                (Wv = second half of Wkv)
  out = softmax(q k^T / sqrt(D) + mask_bias) @ v
  y = out @ Wo.T + bo

Sharding: tensor-parallel over the 16 heads -> 2 heads per core.
Each core computes q/k/v projections for its 2 heads (column-sharded
Wq/Wkv), full attention for those heads, and a partial output
projection (row-sharded Wo).  The host sums the 8 partial outputs.

Device-side layout notes:
 - Everything lives "transposed" ([dim, token]) so no activation
   transposes are needed: host feeds x^T / ctx^T, projections produce
   q^T/k^T directly, scores are computed as S^T = k^T.T @ q^T, and the
   PV matmul consumes P^T directly as the moving operand.
 - The context mask is folded into V (V_masked = V * mask), and V is
   augmented with the mask as a 65th column, so the softmax
   denominator drops out of the PV matmul for free (row 64).
 - All matmuls run in float32r (1 cycle/row vs 4 for fp32; measured
   ~1.3e-4 max rel error on hw).
 - PSUM tiles are 2 banks wide ([128, 1024]); matmuls write 512-wide
   halves, elementwise consumers read the full 1024 in one op.
 - Phases are kept separate (proj / attention / out-proj) so ACT only
   alternates its table function twice per batch element, and the PE
   stream stays dense (HAM stays un-throttled).
"""

import numpy as np

P = 128
B = 2
N = 4096
M = 1024
C = 1024  # DIM == COND_DIM
H = 16
D = 64
HC = 2  # heads per core
CC = C // P  # contraction chunks
NT = N // 1024  # query chunks of 1024
MC = M // P  # kv chunks of 128
EPS = 1e-6

_CACHE = {}


def _build():
    if "nc" in _CACHE:
        return _CACHE["nc"]

    import concourse.bass as bass  # noqa: F401
    import concourse.tile as tile
    from concourse import bacc, mybir

    f32 = mybir.dt.float32
    f32r = mybir.dt.float32r
    AF = mybir.ActivationFunctionType
    MUL = mybir.AluOpType.mult

    nc = bacc.Bacc("TRN2", target_bir_lowering=False, debug=False, num_devices=8)

    xt_d = nc.dram_tensor("xt", [B, C, N], f32r, kind="ExternalInput").ap()
    ctxt_d = nc.dram_tensor("ctxt", [B, C, M], f32r, kind="ExternalInput").ap()
    wqt_d = nc.dram_tensor("wqt", [C, P], f32r, kind="ExternalInput").ap()
    wkt_d = nc.dram_tensor("wkt", [C, P], f32r, kind="ExternalInput").ap()
    wvt_d = nc.dram_tensor("wvt", [C, P], f32r, kind="ExternalInput").ap()
    wot_d = nc.dram_tensor("wot", [P, C], f32r, kind="ExternalInput").ap()
    bq_d = nc.dram_tensor("bq", [P, 1], f32, kind="ExternalInput").ap()
    bk_d = nc.dram_tensor("bk", [P, 1], f32, kind="ExternalInput").ap()
    bv_d = nc.dram_tensor("bv", [P, 1], f32, kind="ExternalInput").ap()
    gq_d = nc.dram_tensor("gq", [HC, P], f32r, kind="ExternalInput").ap()
    gk_d = nc.dram_tensor("gk", [HC, P], f32r, kind="ExternalInput").ap()
    ind2_d = nc.dram_tensor("ind2", [P, HC], f32r, kind="ExternalInput").ap()
    ident_d = nc.dram_tensor("ident", [P, P], f32, kind="ExternalInput").ap()
    mask_d = nc.dram_tensor("maskf", [P, B * MC], f32, kind="ExternalInput").ap()
    y_d = nc.dram_tensor("y", [B, N, C], f32, kind="ExternalOutput").ap()

    with tile.TileContext(nc) as tc:
        with (
            tc.tile_pool(name="consts", bufs=1) as consts,
            tc.tile_pool(name="ctxp", bufs=1) as ctxp,
            tc.tile_pool(name="xp", bufs=2) as xp,
            tc.tile_pool(name="qk", bufs=1) as qk,
            tc.tile_pool(name="work", bufs=2) as work,
            tc.tile_pool(name="ptp", bufs=3) as ptp,
            tc.tile_pool(name="ps", bufs=1, space="PSUM") as ps,
        ):
            wq_sb = consts.tile([P, CC, P], f32r)
            nc.sync.dma_start(wq_sb[:], wqt_d.rearrange("(o p) m -> p o m", p=P))
            wk_sb = consts.tile([P, CC, P], f32r)
            nc.sync.dma_start(wk_sb[:], wkt_d.rearrange("(o p) m -> p o m", p=P))
            wv_sb = consts.tile([P, CC, P], f32r)
            nc.sync.dma_start(wv_sb[:], wvt_d.rearrange("(o p) m -> p o m", p=P))
            wo_sb = consts.tile([P, C], f32r)
            nc.sync.dma_start(wo_sb[:], wot_d[:])
            bq_sb = consts.tile([P, 1], f32)
            nc.sync.dma_start(bq_sb[:], bq_d[:])
            bk_sb = consts.tile([P, 1], f32)
            nc.sync.dma_start(bk_sb[:], bk_d[:])
            bv_sb = consts.tile([P, 1], f32)
            nc.sync.dma_start(bv_sb[:], bv_d[:])
            gq_sb = consts.tile([HC, P], f32r)
            nc.sync.dma_start(gq_sb[:], gq_d[:])
            gk_sb = consts.tile([HC, P], f32r)
            nc.sync.dma_start(gk_sb[:], gk_d[:])
            ind2_sb = consts.tile([P, HC], f32r)
            nc.sync.dma_start(ind2_sb[:], ind2_d[:])
            ident_sb = consts.tile([P, P], f32)
            nc.sync.dma_start(ident_sb[:], ident_d[:])
            mask_sb = consts.tile([P, B * MC], f32)
            nc.sync.dma_start(mask_sb[:], mask_d[:])
            eps_sb = consts.tile([HC, 1], f32)
            nc.vector.memset(eps_sb[:], EPS)

            def rms_norm_chunk(psrc, bias, gind, dst, fw):
                """psrc: [P, fw] psum (fw in {512,1024}) of raw projections
                for 2 heads stacked [64|64]; writes normalized f32r to dst."""
                raw = work.tile([P, 1024], f32, tag="raw", name="raw")[:, :fw]
                nc.vector.tensor_scalar_add(raw, psrc, bias)
                sq = work.tile([P, 1024], f32r, tag="sq", name="sq")[:, :fw]
                nc.vector.tensor_mul(sq, raw, raw)
                ss = ps.tile([HC, 1024], f32, tag="A", bufs=3, name="ss")[:, :fw]
                for half in range(fw // 512):
                    hs = slice(half * 512, (half + 1) * 512)
                    nc.tensor.matmul(
                        ss[:, hs], ind2_sb[:], sq[:, hs], start=True, stop=True
                    )
                srt = work.tile([HC, 1024], f32, tag="srt", name="srt", bufs=1)[:, :fw]
                nc.scalar.activation(srt, ss, AF.Sqrt, scale=1.0 / D, bias=eps_sb[:])
                rstd_f = work.tile([HC, 1024], f32, tag="rstd_f", name="rstd_f", bufs=1)[:, :fw]
                nc.vector.reciprocal_approx_fast(out=rstd_f, in_=srt)
                rstd = work.tile([HC, 1024], f32r, tag="rstd", name="rstd")[:, :fw]
                nc.vector.tensor_copy(rstd, rstd_f)
                bc = ps.tile([P, 1024], f32, tag="A", bufs=3, name="bc")[:, :fw]
                for half in range(fw // 512):
                    hs = slice(half * 512, (half + 1) * 512)
                    nc.tensor.matmul(
                        bc[:, hs], gind[:], rstd[:, hs], start=True, stop=True
                    )
                nc.vector.tensor_mul(dst, raw, bc)

            for b in range(B):
                # ---- KV phase ----
                ctx_sb = ctxp.tile([P, CC, M], f32r, tag="ctx")
                nc.sync.dma_start(
                    ctx_sb[:], ctxt_d[b].rearrange("(o p) m -> p o m", p=P)
                )
                ktn = qk.tile([P, M], f32r, tag="ktn", bufs=2)
                ps_k = ps.tile([P, 1024], f32, tag="A", bufs=3)
                for cc in range(CC):
                    for half in range(2):
                        hs = slice(half * 512, (half + 1) * 512)
                        nc.tensor.matmul(
                            ps_k[:, hs],
                            wk_sb[:, cc],
                            ctx_sb[:, cc, hs],
                            start=(cc == 0),
                            stop=(cc == CC - 1),
                        )
                rms_norm_chunk(ps_k[:], bk_sb, gk_sb, ktn[:], 1024)
                vaug = qk.tile([P, MC, 2 * (D + 1)], f32r, tag="vaug", bufs=2)
                ps_v = ps.tile([P, 1024], f32, tag="A", bufs=3)
                for cc in range(CC):
                    for half in range(2):
                        hs = slice(half * 512, (half + 1) * 512)
                        nc.tensor.matmul(
                            ps_v[:, hs],
                            wv_sb[:, cc],
                            ctx_sb[:, cc, hs],
                            start=(cc == 0),
                            stop=(cc == CC - 1),
                        )
                vt_sb = work.tile([P, M], f32, tag="vt", bufs=1)
                nc.vector.tensor_scalar_add(vt_sb[:], ps_v[:], bv_sb)
                for mc in range(MC):
                    ps_t = ps.tile([P, P], f32, tag="O", bufs=2)
                    nc.tensor.transpose(
                        ps_t[:], vt_sb[:, mc * P : (mc + 1) * P], ident_sb[:]
                    )
                    mcol = mask_sb[:, b * MC + mc : b * MC + mc + 1]
                    nc.vector.tensor_mul(
                        vaug[:, mc, 0:D], ps_t[:, 0:D], mcol.to_broadcast((P, D))
                    )
                    nc.vector.tensor_copy(vaug[:, mc, D : D + 1], mcol)
                    nc.vector.tensor_mul(
                        vaug[:, mc, D + 1 : 2 * D + 1],
                        ps_t[:, D : 2 * D],
                        mcol.to_broadcast((P, D)),
                    )
                    nc.vector.tensor_copy(vaug[:, mc, 2 * D + 1 : 2 * D + 2], mcol)

                # ---- Q projection phase (norms lag one chunk behind) ----
                qtn = qk.tile([P, N], f32r, tag="qtn")
                pending = []
                xt_r = xt_d[b].rearrange("(o p) n -> p o n", p=P)
                for nt in range(NT):
                    nsl = slice(nt * 1024, (nt + 1) * 1024)
                    ps_q = ps.tile([P, 1024], f32, tag="A", bufs=3)
                    for half in range(2):
                        hs = slice(nt * 1024 + half * 512, nt * 1024 + (half + 1) * 512)
                        for cc in range(CC):
                            xt_sb = xp.tile([P, 512], f32r, tag="xt", bufs=8)
                            nc.sync.dma_start(xt_sb[:], xt_r[:, cc, hs])
                            nc.tensor.matmul(
                                ps_q[:, half * 512 : (half + 1) * 512],
                                wq_sb[:, cc],
                                xt_sb[:],
                                start=(cc == 0),
                                stop=(cc == CC - 1),
                            )
                    pending.append((ps_q, nsl))
                    if len(pending) >= 2:
                        pq, pn = pending.pop(0)
                        rms_norm_chunk(pq[:], bq_sb, gq_sb, qtn[:, pn], 1024)
                for pq, pn in pending:
                    rms_norm_chunk(pq[:], bq_sb, gq_sb, qtn[:, pn], 1024)

                # ---- Attention phase (n in chunks of 512) ----
                outtn = qk.tile([P, N], f32r, tag="outtn")
                for nt in range(N // 512):
                    nsl = slice(nt * 512, (nt + 1) * 512)
                    ps_o = [
                        ps.tile([D + 1, 512], f32, tag="O", bufs=2, name=f"ps_o{h}")
                        for h in range(2)
                    ]
                    for mc in range(MC):
                        msl = slice(mc * P, (mc + 1) * P)
                        ps_s = ps.tile([P, 1024], f32, tag="A", bufs=3)
                        for h in range(2):
                            hsl = slice(D * h, D * (h + 1))
                            nc.tensor.matmul(
                                ps_s[:, h * 512 : (h + 1) * 512],
                                ktn[hsl, msl],
                                qtn[hsl, nsl],
                                start=True,
                                stop=True,
                            )
                        pt = ptp.tile([P, 1024], f32r, tag="pt")
                        nc.scalar.activation(pt[:], ps_s[:], AF.Exp)
                        for h in range(2):
                            nc.tensor.matmul(
                                ps_o[h][:],
                                vaug[:, mc, h * (D + 1) : (h + 1) * (D + 1)],
                                pt[:, h * 512 : (h + 1) * 512],
                                start=(mc == 0),
                                stop=(mc == MC - 1),
                            )
                    for h in range(2):
                        denrow = work.tile([1, 512], f32, tag="denrow", bufs=1)
                        nc.vector.tensor_copy(denrow[:], ps_o[h][D : D + 1, :])
                        recrow = work.tile([1, 512], f32, tag="recrow", bufs=1)
                        nc.vector.reciprocal_approx_fast(out=recrow[:], in_=denrow[:])
                        bcn = work.tile([D, 512], f32, tag="bcn", bufs=1)
                        nc.gpsimd.partition_broadcast(bcn[:], recrow[:])
                        nc.vector.scalar_tensor_tensor(
                            out=outtn[D * h : D * (h + 1), nsl],
                            in0=ps_o[h][0:D, :],
                            scalar=1.0,
                            in1=bcn[:],
                            op0=MUL,
                            op1=MUL,
                        )

                # ---- Output projection phase ----
                for tc_ in range(N // P):
                    y_sb = work.tile([P, C], f32, tag="ysb")
                    ps_y = ps.tile([P, 1024], f32, tag="A", bufs=3)
                    for ec in range(2):
                        nc.tensor.matmul(
                            ps_y[:, ec * 512 : (ec + 1) * 512],
                            outtn[:, tc_ * P : (tc_ + 1) * P],
                            wo_sb[:, ec * 512 : (ec + 1) * 512],
                            start=True,
                            stop=True,
                        )
                    nc.scalar.activation(y_sb[:, 0:512], ps_y[:, 0:512], AF.Copy)
                    nc.vector.tensor_copy(y_sb[:, 512:1024], ps_y[:, 512:1024])
                    nc.sync.dma_start(y_d[b, tc_ * P : (tc_ + 1) * P, :], y_sb[:])

    nc.compile()
    _CACHE["nc"] = nc
    return nc


def _make_in_maps(x, context, context_mask, Wq, bq, Wkv, bkv, gq, gk, Wo, bo):
    f32 = np.float32
    xt = np.ascontiguousarray(np.transpose(x, (0, 2, 1)), dtype=f32)
    ctxt = np.ascontiguousarray(np.transpose(context, (0, 2, 1)), dtype=f32)
    # maskf[p, b*MC + mc] = mask[b, mc*128 + p]
    maskf = np.ascontiguousarray(
        np.transpose(
            np.asarray(context_mask, dtype=f32).reshape(B, MC, P), (2, 0, 1)
        ).reshape(P, B * MC)
    )
    ident = np.eye(P, dtype=f32)
    ind2 = np.zeros((P, HC), dtype=f32)
    for h in range(HC):
        ind2[D * h : D * (h + 1), h] = 1.0

    in_maps = []
    for c in range(8):
        hs = slice(P * c, P * (c + 1))
        gq_c = np.zeros((HC, P), dtype=f32)
        gk_c = np.zeros((HC, P), dtype=f32)
        for h in range(HC):
            gq_c[h, D * h : D * (h + 1)] = gq[HC * c + h] * (1.0 / np.sqrt(D))
            gk_c[h, D * h : D * (h + 1)] = gk[HC * c + h]
        in_maps.append(
            {
                "xt": xt,
                "ctxt": ctxt,
                "wqt": np.ascontiguousarray(Wq[hs].T, dtype=f32),
                "wkt": np.ascontiguousarray(Wkv[hs].T, dtype=f32),
                "wvt": np.ascontiguousarray(Wkv[C + P * c : C + P * (c + 1)].T, dtype=f32),
                "wot": np.ascontiguousarray(Wo[:, hs].T, dtype=f32),
                "bq": np.asarray(bq[hs], dtype=f32).reshape(P, 1),
                "bk": np.asarray(bkv[hs], dtype=f32).reshape(P, 1),
                "bv": np.asarray(bkv[C + P * c : C + P * (c + 1)], dtype=f32).reshape(P, 1),
                "gq": gq_c,
                "gk": gk_c,
                "ind2": ind2,
                "ident": ident,
                "maskf": maskf,
            }
        )
    return in_maps


def _run(in_maps, **spmd_kwargs):
    from concourse import bass_utils

    nc = _build()
    return bass_utils.run_bass_kernel_spmd(
        nc, in_maps, core_ids=list(range(8)), **spmd_kwargs
    )


def kernel(x, context, context_mask, Wq, bq, Wkv, bkv, gq, gk, Wo, bo):
    in_maps = _make_in_maps(
        x, context, context_mask, Wq, bq, Wkv, bkv, gq, gk, Wo, bo
    )
    res = _run(in_maps)
    y = np.zeros((B, N, C), dtype=np.float64)
    for c in range(8):
        y += res.results[c]["y"]
    y += np.asarray(bo, dtype=np.float64)
    return y.astype(np.float32)



# revision 23
# speedup vs baseline: 1.9716x; 1.9716x over previous
"""Cross-attention kernel for Trainium2, sharded over 8 NeuronCores.

Problem (hardcoded shapes): B=2, N=4096, M=1024, DIM=1024, H=16, D=64.
  q = rms_norm(x @ Wq.T + bq)        per-head, gamma gq, eps 1e-6
  k = rms_norm(ctx @ Wk.T + bk)      (Wk = first half of Wkv)
  v = ctx @ Wv.T + bv                (Wv = second half of Wkv)
  out = softmax(q k^T / sqrt(D) + mask_bias) @ v
  y = out @ Wo.T + bo

Sharding: 2 batches x 4 head-groups -> 8 cores.  Core c handles batch
c//4 and heads [4*(c%4), 4*(c%4)+4).  Each core computes q/k/v
projections for its 4 heads on its batch, attention, and a partial
output projection (row-sharded Wo).  Host sums the 4 partials per
batch and adds bo.

Key device-side choices:
 - Context is COMPACTED on the host: only valid (mask=1) tokens are
   kept, zero-padded to a multiple of 128 (M_pad).  Padding is exact:
   V rows are zeroed via a mask column, and the softmax denominator is
   computed with the mask column as the matmul stationary, so padded
   rows contribute exactly 0 to numerator and denominator.
 - All activations/weights are fp16 (fp32 PSUM accumulation).  exp(s)
   is bounded by e^8 (|q|=|k|=8 after rms norm, scale 1/8) so fp16
   probabilities cannot overflow.
 - Attention matmuls are PE-tile-packed: scores run as row-tiled
   (K=64) head pairs, PV as col-tiled (M=64) head pairs, and the four
   denominators as 4-way col-tiled M=1 matmuls -> full 128x128 array
   utilization.
 - V is projected directly into [m, d] layout (ctx chunk stationary,
   Wv^T moving) so no PE transposes are needed anywhere.
 - The main loop is software-pipelined: per 512-query block, the
   Q-projection/rms-norm of block nt is interleaved instruction-by-
   instruction with the attention of block nt-1 (PV lagging scores by
   one m-chunk) so the PE never waits on the ACT exp chain and the
   HAM clock gate stays un-throttled.
"""

import numpy as np

P = 128
B = 2
N = 4096
M = 1024
C = 1024  # DIM == COND_DIM
H = 16
D = 64
HC = 4  # heads per core
VD = HC * D  # 256 v/q/k dims per core
CC = C // P  # contraction chunks (8)
NT = N // 512  # query blocks of 512 (8)
QT = 2  # qdim tiles of 128 (VD / P)
EPS = 1e-6

_CACHE = {}


def _build(MC, dbg=False):
    """Build the kernel for MC context chunks of 128 (M_pad = 128*MC)."""
    key = ("nc", MC, dbg)
    if key in _CACHE:
        return _CACHE[key]

    import concourse.bass as bass  # noqa: F401
    import concourse.tile as tile
    from concourse import bacc, mybir

    f32 = mybir.dt.float32
    f16 = mybir.dt.float16
    AF = mybir.ActivationFunctionType
    MUL = mybir.AluOpType.mult
    MP = MC * P  # padded context length

    nc = bacc.Bacc("TRN2", target_bir_lowering=False, debug=False, num_devices=8)

    xt_d = nc.dram_tensor("xt", [C, N], f16, kind="ExternalInput").ap()
    ctxt_d = nc.dram_tensor("ctxt", [C, MP], f16, kind="ExternalInput").ap()
    wqt_d = nc.dram_tensor("wqt", [C, VD], f16, kind="ExternalInput").ap()
    wkt_d = nc.dram_tensor("wkt", [C, VD], f16, kind="ExternalInput").ap()
    wvt_d = nc.dram_tensor("wvt", [C, VD], f16, kind="ExternalInput").ap()
    wot_d = nc.dram_tensor("wot", [VD, C], f16, kind="ExternalInput").ap()
    bq_d = nc.dram_tensor("bq2", [P, QT], f32, kind="ExternalInput").ap()
    bk_d = nc.dram_tensor("bk2", [P, QT], f32, kind="ExternalInput").ap()
    bvbm_d = nc.dram_tensor("bvbm", [P, MC, VD], f16, kind="ExternalInput").ap()
    gqi_d = nc.dram_tensor("gqi", [P, P], f16, kind="ExternalInput").ap()
    gki_d = nc.dram_tensor("gki", [P, P], f16, kind="ExternalInput").ap()
    ind2_d = nc.dram_tensor("ind2", [P, 2], f16, kind="ExternalInput").ap()
    mask16_d = nc.dram_tensor("mask16", [P, MC], f16, kind="ExternalInput").ap()
    mask32_d = nc.dram_tensor("mask32", [P, MC], f32, kind="ExternalInput").ap()
    y_d = nc.dram_tensor("y", [N, C], f16, kind="ExternalOutput").ap()
    if dbg:
        dbg_ktn = nc.dram_tensor("dbg_ktn", [QT, P, MC * P], f16, kind="ExternalOutput").ap()
        dbg_vt = nc.dram_tensor("dbg_vt", [MC, P, VD], f16, kind="ExternalOutput").ap()
        dbg_qtn = nc.dram_tensor("dbg_qtn", [NT, QT, P, 512], f16, kind="ExternalOutput").ap()
        dbg_rec = nc.dram_tensor("dbg_rec", [NT, P, 512], f16, kind="ExternalOutput").ap()
        dbg_pt = nc.dram_tensor("dbg_pt", [MC, 2, P, 1024], f16, kind="ExternalOutput").ap()
        dbg_outtn = nc.dram_tensor("dbg_outtn", [QT, P, N], f16, kind="ExternalOutput").ap()

    with tile.TileContext(nc) as tc:
        with (
            tc.tile_pool(name="consts", bufs=1) as consts,
            tc.tile_pool(name="xpool", bufs=1) as xpool,
            tc.tile_pool(name="kv", bufs=1) as kvp,
            tc.tile_pool(name="work", bufs=2) as work,
            tc.tile_pool(name="ptp", bufs=5) as ptp,
            tc.tile_pool(name="outp", bufs=1) as outp,
            # PSUM: "big" = [128,1024] 2-bank tiles (scores pairs, KV proj,
            # out-proj), bufs=2 -> 4 banks.  "acc" = [128,512] 1-bank tiles
            # (pv01, pv23, den persist per nt), bufs=3 -> 3 banks.
            # "mi" = [128,512] 1-bank (Qproj halves / ss / bc), bufs=1.
            tc.tile_pool(name="big", bufs=2, space="PSUM") as bigp,
            tc.tile_pool(name="acc", bufs=3, space="PSUM") as accp,
            tc.tile_pool(name="mi", bufs=1, space="PSUM") as mip,
        ):
            # ---- constants / weights ----
            wq_sb = consts.tile([P, CC, VD], f16)
            nc.sync.dma_start(wq_sb[:], wqt_d.rearrange("(o p) m -> p o m", p=P))
            wk_sb = consts.tile([P, CC, VD], f16)
            nc.sync.dma_start(wk_sb[:], wkt_d.rearrange("(o p) m -> p o m", p=P))
            wv_sb = consts.tile([P, CC, VD], f16)
            nc.sync.dma_start(wv_sb[:], wvt_d.rearrange("(o p) m -> p o m", p=P))
            wo_sb = consts.tile([P, QT, C], f16)
            nc.sync.dma_start(wo_sb[:], wot_d.rearrange("(o p) m -> p o m", p=P))
            bq_sb = consts.tile([P, QT], f32)
            nc.sync.dma_start(bq_sb[:], bq_d[:])
            bk_sb = consts.tile([P, QT], f32)
            nc.sync.dma_start(bk_sb[:], bk_d[:])
            bvbm_sb = consts.tile([P, MC, VD], f16)
            nc.sync.dma_start(bvbm_sb[:], bvbm_d[:])
            gqi_sb = consts.tile([P, P], f16)
            nc.sync.dma_start(gqi_sb[:], gqi_d[:])
            gki_sb = consts.tile([P, P], f16)
            nc.sync.dma_start(gki_sb[:], gki_d[:])
            ind2_sb = consts.tile([P, 2], f16)
            nc.sync.dma_start(ind2_sb[:], ind2_d[:])
            m16_sb = consts.tile([P, MC], f16)
            nc.sync.dma_start(m16_sb[:], mask16_d[:])
            m32_sb = consts.tile([P, MC], f32)
            nc.sync.dma_start(m32_sb[:], mask32_d[:])
            eps_sb = consts.tile([P, 1], f32)
            nc.vector.memset(eps_sb[:], EPS)
            ones64_sb = consts.tile([P, 64], f16)
            nc.vector.memset(ones64_sb[:], 1.0)

            # full x resident in SBUF (fp16, 64KB/partition)
            ctx_sb = xpool.tile([P, CC, MP], f16)
            nc.sync.dma_start(ctx_sb[:], ctxt_d.rearrange("(o p) m -> p o m", p=P))
            xt_sb = xpool.tile([P, CC, N], f16)
            nc.sync.dma_start(xt_sb[:], xt_d.rearrange("(o p) n -> p o n", p=P))

            # ================= KV phase =================
            # K projection: out [kdim, m] (2 tiles of 128 kdims)
            ktn = [kvp.tile([P, MP], f16, name=f"ktn{t}") for t in range(QT)]
            kraw = [kvp.tile([P, MP], f16, name=f"kraw{t}") for t in range(QT)]
            for t in range(QT):
                ps_k = bigp.tile([P, 1024], f32, tag="big")
                for cc in range(CC):
                    for ms in range(0, MP, 512):
                        me = min(ms + 512, MP)
                        nc.tensor.matmul(
                            ps_k[:, ms:me],
                            wk_sb[:, cc, t * P : (t + 1) * P],
                            ctx_sb[:, cc, ms:me],
                            start=(cc == 0),
                            stop=(cc == CC - 1),
                        )
                nc.vector.tensor_scalar_add(
                    kraw[t][:], ps_k[:, :MP], bk_sb[:, t : t + 1]
                )
                sq = work.tile([P, MP], f16, tag="ksq", name="ksq")
                nc.vector.tensor_mul(sq[:], kraw[t][:], kraw[t][:])
                rsl = slice(32 * t, 32 * t + 2)
                ps_ss = bigp.tile([P, 1024], f32, tag="big", name=f"kss{t}")
                for ms in range(0, MP, 512):
                    me = min(ms + 512, MP)
                    nc.tensor.matmul(
                        ps_ss[rsl, ms:me],
                        ind2_sb[:],
                        sq[:, ms:me],
                        start=True,
                        stop=True,
                    )
                # rsqrt(mean_sq + eps) = Exp(-0.5 * Ln(ss/D + eps)); Ln and
                # Exp share one ACT table set so no table switches ever.
                srt = work.tile([34, MP], f32, tag="ksrt", name="ksrt", bufs=1)
                nc.scalar.activation(
                    srt[rsl, :], ps_ss[rsl, :MP], AF.Ln, scale=1.0 / D,
                    bias=eps_sb[rsl, :],
                )
                rstd16 = work.tile([34, MP], f16, tag="krstd16", name="krstd16")
                nc.scalar.activation(rstd16[rsl, :], srt[rsl, :], AF.Exp, scale=-0.5)
                ps_bc = bigp.tile([P, 1024], f32, tag="big", name=f"kbc{t}")
                for ms in range(0, MP, 512):
                    me = min(ms + 512, MP)
                    nc.tensor.matmul(
                        ps_bc[:, ms:me],
                        gki_sb[rsl, :],
                        rstd16[rsl, ms:me],
                        start=True,
                        stop=True,
                    )
                nc.vector.tensor_mul(ktn[t][:], kraw[t][:], ps_bc[:, :MP])
                if dbg:
                    nc.sync.dma_start(dbg_ktn[t], ktn[t][:])

            # V projection directly in [m, vdim] layout + bias + mask
            vt = []
            for mc in range(MC):
                ps_v = mip.tile([P, 512], f32, tag="mi", name=f"v{mc}")
                for cc in range(CC):
                    nc.tensor.matmul(
                        ps_v[:, 0:VD],
                        ctx_sb[:, cc, mc * P : (mc + 1) * P],
                        wv_sb[:, cc, :],
                        start=(cc == 0),
                        stop=(cc == CC - 1),
                    )
                vtile = kvp.tile([P, VD], f16, name=f"vt{mc}")
                # v = vproj * maskcol + (bv * maskcol)
                nc.vector.scalar_tensor_tensor(
                    out=vtile[:],
                    in0=ps_v[:, 0:VD],
                    scalar=m32_sb[:, mc : mc + 1],
                    in1=bvbm_sb[:, mc, :],
                    op0=MUL,
                    op1=mybir.AluOpType.add,
                )
                vt.append(vtile)
                if dbg:
                    nc.sync.dma_start(dbg_vt[mc], vtile[:])

            # ================= main pipelined loop =================
            # state carried between iterations
            qtn_tiles = [None, None]  # current nt's normalized q (2 tiles)
            prev = None  # (qtn tiles, ) of previous nt awaiting attention

            def issue_qproj_half(nt, t, dst_raw):
                """Q projection for qdim tile t of block nt -> raw16."""
                nsl = slice(nt * 512, (nt + 1) * 512)
                ps_q = mip.tile([P, 512], f32, tag="mi", name=f"q{nt}_{t}")
                for cc in range(CC):
                    nc.tensor.matmul(
                        ps_q[:],
                        wq_sb[:, cc, t * P : (t + 1) * P],
                        xt_sb[:, cc, nsl],
                        start=(cc == 0),
                        stop=(cc == CC - 1),
                    )
                nc.vector.tensor_scalar_add(dst_raw[:], ps_q[:], bq_sb[:, t : t + 1])

            # Attention for block `nt` uses tiles from `state`:
            #   qtn (2 tiles [128,512] f16), produces outtn via pv/den.
            outtn = [
                outp.tile([P, N], f16, name=f"outtn{t}") for t in range(QT)
            ]

            for step in range(NT + 1):
                do_q = step < NT
                do_attn = step > 0
                ant = step - 1  # attention block index

                # ---- rms/proj state for this step's Q block ----
                if do_q:
                    raw16 = [
                        work.tile([P, 512], f16, tag=f"qraw{t}", name=f"qraw{t}")
                        for t in range(QT)
                    ]
                    sq16 = [
                        work.tile([P, 512], f16, tag=f"qsq{t}", name=f"qsq{t}")
                        for t in range(QT)
                    ]
                    qtn_tiles = [
                        work.tile([P, 512], f16, tag=f"qtn{t}", name=f"qtn{t}")
                        for t in range(QT)
                    ]

                if do_attn:
                    aqtn = prev
                    ps_pv = [
                        accp.tile([P, 512], f32, tag="acc", name=f"pv{pr}")
                        for pr in range(2)
                    ]
                    ps_den = accp.tile([P, 512], f32, tag="acc", name="den")
                    pt_tiles = []

                # interleave: for each mc, issue scores(mc), exp(mc),
                # then a slice of Q-proj work, then pv/den(mc-1).
                n_mc = MC if do_attn else 0

                def qwork_slice(i):
                    """Issue the i-th slice of this step's Q-proj/rms work."""
                    if not do_q:
                        return
                    if i == 0:
                        issue_qproj_half(step, 0, raw16[0])
                        nc.vector.tensor_mul(sq16[0][:], raw16[0][:], raw16[0][:])
                    elif i == 1:
                        issue_qproj_half(step, 1, raw16[1])
                        nc.vector.tensor_mul(sq16[1][:], raw16[1][:], raw16[1][:])
                    elif i == 2:
                        # ss col-tiled pair: rows 0:2 (tile0) and 32:34 (tile1)
                        ps_ss = mip.tile([P, 512], f32, tag="mi", name="qss")
                        nc.tensor.matmul(
                            ps_ss[0:2, :], ind2_sb[:], sq16[0][:],
                            start=True, stop=True,
                        )
                        nc.tensor.matmul(
                            ps_ss[32:34, :], ind2_sb[:], sq16[1][:],
                            start=True, stop=True,
                        )
                        srt = work.tile([34, 512], f32, tag="qsrt", name="qsrt")
                        nc.scalar.activation(
                            srt[:], ps_ss[0:34, :], AF.Ln, scale=1.0 / D,
                            bias=eps_sb[0:34, :],
                        )
                        r16 = work.tile([34, 512], f16, tag="qr16", name="qr16")
                        nc.scalar.activation(r16[:], srt[:], AF.Exp, scale=-0.5)
                        self_r16[0] = r16
                    elif i in (3, 4):
                        t = i - 3
                        r16 = self_r16[0]
                        ps_bc = mip.tile([P, 512], f32, tag="mi", name=f"qbc{t}")
                        nc.tensor.matmul(
                            ps_bc[:],
                            gqi_sb[32 * t : 32 * t + 2, :],
                            r16[32 * t : 32 * t + 2, :],
                            start=True,
                            stop=True,
                        )
                        nc.vector.tensor_mul(qtn_tiles[t][:], raw16[t][:], ps_bc[:])
                        if dbg:
                            nc.sync.dma_start(dbg_qtn[step, t], qtn_tiles[t][:])

                self_r16 = [None]

                if not do_attn:
                    for i in range(5):
                        qwork_slice(i)
                else:
                    ansl = slice(ant * 512, (ant + 1) * 512)
                    qi = 0
                    for mc in range(n_mc + 1):
                        if mc < n_mc:
                            # scores for both head pairs, row-tiled (K=64)
                            pt_pair = []
                            for pr in range(2):
                                ps_s = bigp.tile(
                                    [P, 1024], f32, tag="big", name=f"s{mc}_{pr}"
                                )
                                kt = ktn[pr]
                                qt = aqtn[pr]
                                msl = slice(mc * P, (mc + 1) * P)
                                nc.tensor.matmul(
                                    ps_s[:, 0:512], kt[0:64, msl], qt[0:64, :],
                                    start=True, stop=True,
                                )
                                nc.tensor.matmul(
                                    ps_s[:, 512:1024], kt[64:128, msl], qt[64:128, :],
                                    start=True, stop=True,
                                )
                                pt = ptp.tile([P, 1024], f16, tag="pt")
                                nc.scalar.activation(pt[:], ps_s[:], AF.Exp)
                                if dbg and ant == 0:
                                    nc.sync.dma_start(dbg_pt[mc, pr], pt[:])
                                pt_pair.append(pt)
                            pt_tiles.append(pt_pair)
                        # a slice of Q work between scores and pv
                        if qi < 5:
                            qwork_slice(qi)
                            qi += 1
                        # pv/den for previous mc (lag 1)
                        pmc = mc - 1
                        if 0 <= pmc:
                            pt_pair = pt_tiles[pmc]
                            for pr in range(2):
                                pt = pt_pair[pr]
                                for hh in range(2):
                                    h = 2 * pr + hh
                                    nc.tensor.matmul(
                                        ps_pv[pr][64 * hh : 64 * hh + 64, :],
                                        vt[pmc][:, 64 * h : 64 * h + 64],
                                        pt[:, 512 * hh : 512 * hh + 512],
                                        start=(pmc == 0),
                                        stop=(pmc == MC - 1),
                                    )
                            for pr in range(2):
                                pt = pt_pair[pr]
                                for hh in range(2):
                                    h = 2 * pr + hh
                                    nc.tensor.matmul(
                                        ps_den[32 * h : 32 * h + 1, :],
                                        m16_sb[:, pmc : pmc + 1],
                                        pt[:, 512 * hh : 512 * hh + 512],
                                        start=(pmc == 0),
                                        stop=(pmc == MC - 1),
                                        tile_position=(0, 32 * h),
                                    )
                    while qi < 5:
                        qwork_slice(qi)
                        qi += 1

                    # ---- normalize -> outtn ----
                    # ps_den holds den/256 (mask stationary is 1/256), so the
                    # fp16 reciprocal 256/den stays in normal fp16 range; the
                    # stt scalar 1/256 compensates exactly.
                    rd32 = work.tile([P, 512], f32, tag="rd32", name="rd32")
                    nc.vector.reciprocal_approx_fast(
                        out=rd32[0:97, :], in_=ps_den[0:97, :]
                    )
                    rd16 = work.tile([P, 512], f16, tag="rd16", name="rd16")
                    nc.vector.tensor_copy(rd16[0:97, :], rd32[0:97, :])
                    if dbg:
                        nc.sync.dma_start(dbg_rec[ant], rd16[:])
                    for pr in range(2):
                        # broadcast each head's recip row across 64 partitions
                        # with a K=1 matmul (ones column stationary)
                        ps_bcn = mip.tile([P, 512], f32, tag="mi", name=f"bcn{pr}")
                        for hh in range(2):
                            h = 2 * pr + hh
                            nc.tensor.matmul(
                                ps_bcn[64 * hh : 64 * hh + 64, :],
                                ones64_sb[32 * h : 32 * h + 1, :],
                                rd16[32 * h : 32 * h + 1, :],
                                start=True,
                                stop=True,
                                tile_position=(32 * h, 64 * hh),
                            )
                        bcn_sb = work.tile(
                            [P, 512], f16, tag=f"bcn{pr}", name=f"bcn{pr}"
                        )
                        nc.vector.tensor_copy(bcn_sb[:], ps_bcn[:])
                        nc.vector.scalar_tensor_tensor(
                            out=outtn[pr][:, ansl],
                            in0=ps_pv[pr][:],
                            scalar=1.0 / 256.0,
                            in1=bcn_sb[:],
                            op0=MUL,
                            op1=MUL,
                        )

                prev = qtn_tiles

            if dbg:
                for t in range(QT):
                    nc.sync.dma_start(dbg_outtn[t], outtn[t][:])

            # ================= output projection =================
            for tcn in range(N // P):
                ps_y = bigp.tile([P, 1024], f32, tag="big", name="ps_y")
                tsl = slice(tcn * P, (tcn + 1) * P)
                for half in range(2):
                    ysl = slice(half * 512, (half + 1) * 512)
                    for t in range(QT):
                        nc.tensor.matmul(
                            ps_y[:, ysl],
                            outtn[t][:, tsl],
                            wo_sb[:, t, ysl],
                            start=(t == 0),
                            stop=(t == QT - 1),
                        )
                y_sb = work.tile([P, C], f16, tag="ysb", name="ysb")
                if tcn % 2 == 0:
                    nc.scalar.activation(y_sb[:, 0:512], ps_y[:, 0:512], AF.Copy)
                    nc.vector.tensor_copy(y_sb[:, 512:1024], ps_y[:, 512:1024])
                else:
                    nc.vector.tensor_copy(y_sb[:, 0:512], ps_y[:, 0:512])
                    nc.scalar.activation(y_sb[:, 512:1024], ps_y[:, 512:1024], AF.Copy)
                nc.sync.dma_start(y_d[tsl, :], y_sb[:])

    nc.compile()
    _CACHE[key] = nc
    return nc


def _prep(x, context, context_mask, Wq, bq, Wkv, bkv, gq, gk, Wo, bo):
    """Host-side: compaction, transposes, per-core weight slices."""
    f16 = np.float16
    f32 = np.float32
    mask = np.asarray(context_mask)
    idxs = [np.nonzero(mask[b])[0] for b in range(B)]
    mv = [len(ix) for ix in idxs]
    MC = max(1, (max(mv) + P - 1) // P)
    MP = MC * P

    # compacted, padded, transposed context per batch (fp16)
    ctxt = []
    for b in range(B):
        cc = np.zeros((MP, C), dtype=f32)
        cc[: mv[b]] = np.asarray(context[b], dtype=f32)[idxs[b]]
        ctxt.append(np.ascontiguousarray(cc.T, dtype=f16))

    # mask columns [128, MC] per batch
    m32 = []
    for b in range(B):
        m = np.zeros((MP,), dtype=f32)
        m[: mv[b]] = 1.0
        m32.append(np.ascontiguousarray(m.reshape(MC, P).T))

    xt = [
        np.ascontiguousarray(np.asarray(x[b], dtype=f32).T, dtype=f16)
        for b in range(B)
    ]

    ind2 = np.zeros((P, 2), dtype=f16)
    ind2[0:64, 0] = 1.0
    ind2[64:128, 1] = 1.0

    Wq = np.asarray(Wq, dtype=f32)
    Wkv = np.asarray(Wkv, dtype=f32)
    Wo = np.asarray(Wo, dtype=f32)
    bq = np.asarray(bq, dtype=f32)
    bkv = np.asarray(bkv, dtype=f32)
    gq = np.asarray(gq, dtype=f32)
    gk = np.asarray(gk, dtype=f32)

    in_maps = []
    for c in range(8):
        bi, hg = c // 4, c % 4
        hs = slice(VD * hg, VD * (hg + 1))  # 256 dims for 4 heads
        heads = [hg * HC + j for j in range(HC)]

        gqi = np.zeros((P, P), dtype=f16)
        gki = np.zeros((P, P), dtype=f16)
        for t in range(QT):
            for j in range(2):
                h = heads[2 * t + j]
                gqi[32 * t + j, 64 * j : 64 * j + 64] = (
                    gq[h] * (1.0 / np.sqrt(D))
                ).astype(f16)
                gki[32 * t + j, 64 * j : 64 * j + 64] = gk[h].astype(f16)

        bv = bkv[C + VD * hg : C + VD * (hg + 1)]
        bvbm = np.zeros((P, MC, VD), dtype=f16)
        for mc in range(MC):
            bvbm[:, mc, :] = (
                m32[bi][:, mc : mc + 1] * bv[None, :]
            ).astype(f16)

        in_maps.append(
            {
                "xt": xt[bi],
                "ctxt": ctxt[bi],
                "wqt": np.ascontiguousarray(Wq[hs].T, dtype=f16),
                "wkt": np.ascontiguousarray(Wkv[hs].T, dtype=f16),
                "wvt": np.ascontiguousarray(
                    Wkv[C + VD * hg : C + VD * (hg + 1)].T, dtype=f16
                ),
                "wot": np.ascontiguousarray(Wo[:, hs].T, dtype=f16),
                "bq2": np.ascontiguousarray(
                    bq[hs].reshape(QT, P).T, dtype=f32
                ),
                "bk2": np.ascontiguousarray(
                    bkv[hs].reshape(QT, P).T, dtype=f32
                ),
                "bvbm": bvbm,
                "gqi": gqi,
                "gki": gki,
                "ind2": ind2,
                "mask16": (m32[bi] / 256.0).astype(f16),
                "mask32": m32[bi],
            }
        )
    return in_maps, MC


def _run(in_maps, MC, **spmd_kwargs):
    from concourse import bass_utils

    nc = _build(MC)
    return bass_utils.run_bass_kernel_spmd(
        nc, in_maps, core_ids=list(range(8)), **spmd_kwargs
    )


def kernel(x, context, context_mask, Wq, bq, Wkv, bkv, gq, gk, Wo, bo):
    in_maps, MC = _prep(
        x, context, context_mask, Wq, bq, Wkv, bkv, gq, gk, Wo, bo
    )
    res = _run(in_maps, MC)
    y = np.zeros((B, N, C), dtype=np.float64)
    for c in range(8):
        y[c // 4] += res.results[c]["y"].astype(np.float64)
    y += np.asarray(bo, dtype=np.float64)[None, None, :]
    return y.astype(np.float32)


# revision 25
# speedup vs baseline: 2.2928x; 1.1629x over previous
"""Cross-attention kernel for Trainium2, sharded over 8 NeuronCores.

Problem (hardcoded shapes): B=2, N=4096, M=1024, DIM=1024, H=16, D=64.
  q = rms_norm(x @ Wq.T + bq)        per-head, gamma gq, eps 1e-6
  k = rms_norm(ctx @ Wk.T + bk)      (Wk = first half of Wkv)
  v = ctx @ Wv.T + bv                (Wv = second half of Wkv)
  out = softmax(q k^T / sqrt(D) + mask_bias) @ v
  y = out @ Wo.T + bo

Sharding: 2 batches x 4 head-groups -> 8 cores.  Core c handles batch
c//4 and heads [4*(c%4), 4*(c%4)+4).  Each core computes q/k/v
projections for its 4 heads on its batch, attention, and a partial
output projection (row-sharded Wo).  Host sums the 4 partials per
batch and adds bo.

Key device-side choices:
 - Context is COMPACTED on the host: only valid (mask=1) tokens are
   kept, zero-padded to a multiple of 128 (M_pad).  Padding is exact:
   V rows are zeroed via a mask column, and the softmax denominator is
   computed with the mask column as the matmul stationary, so padded
   rows contribute exactly 0 to numerator and denominator.
 - All activations/weights are fp16 (fp32 PSUM accumulation).  exp(s)
   is bounded by e^8 (|q|=|k|=8 after rms norm, scale 1/8) so fp16
   probabilities cannot overflow.
 - Attention matmuls are PE-tile-packed: scores run as row-tiled
   (K=64) head pairs, PV as col-tiled (M=64) head pairs, and the four
   denominators as 4-way col-tiled M=1 matmuls -> full 128x128 array
   utilization.
 - V is projected directly into [m, d] layout (ctx chunk stationary,
   Wv^T moving) so no PE transposes are needed anywhere.
 - The main loop is software-pipelined: per 512-query block, the
   Q-projection/rms-norm of block nt is interleaved instruction-by-
   instruction with the attention of block nt-1 (PV lagging scores by
   one m-chunk) so the PE never waits on the ACT exp chain and the
   HAM clock gate stays un-throttled.
"""

import numpy as np

P = 128
B = 2
N = 4096
M = 1024
C = 1024  # DIM == COND_DIM
H = 16
D = 64
HC = 4  # heads per core
VD = HC * D  # 256 v/q/k dims per core
CC = C // P  # contraction chunks (8)
NT = N // 512  # query blocks of 512 (8)
QT = 2  # qdim tiles of 128 (VD / P)
EPS = 1e-6

_CACHE = {}


def _build(MC, dbg=False):
    """Build the kernel for MC context chunks of 128 (M_pad = 128*MC)."""
    key = ("nc", MC, dbg)
    if key in _CACHE:
        return _CACHE[key]

    import concourse.bass as bass  # noqa: F401
    import concourse.tile as tile
    from concourse import bacc, mybir

    f32 = mybir.dt.float32
    f16 = mybir.dt.float16
    AF = mybir.ActivationFunctionType
    MUL = mybir.AluOpType.mult
    MP = MC * P  # padded context length

    nc = bacc.Bacc("TRN2", target_bir_lowering=False, debug=False, num_devices=8)

    # All ACT functions used here (Exp, Ln, Copy, Identity) live in the
    # single table set "natural_log_exp_and_others".  The default set
    # assignment pass picks a different set per function and thrashes
    # ~20 ACT_TABLE_LOADs (~1.3us each); restrict the candidate list so
    # the fixpoint pass hoists ONE load to kernel entry.
    import types as _types
    import bass_rust as _bass_rust
    from concourse.hw_specs import get_activation_tables as _gat

    def _act_loads_single_set(self):
        has_act = any(
            isinstance(i, mybir.InstActivation)
            for b in self.main_func.blocks
            for i in b.instructions
        )
        if not has_act:
            return
        tables = list(_gat(self.m.arch).items())
        keep = "natural_log_exp_and_others"
        filtered = [(n, (set(fns) if n == keep else set())) for n, fns in tables]
        _bass_rust.insert_act_table_loads(self, filtered)

    nc.insert_act_table_loads = _types.MethodType(_act_loads_single_set, nc)

    xt_d = nc.dram_tensor("xt", [C, N], f16, kind="ExternalInput").ap()
    ctxt_d = nc.dram_tensor("ctxt", [C, MP], f16, kind="ExternalInput").ap()
    wqt_d = nc.dram_tensor("wqt", [C, VD], f16, kind="ExternalInput").ap()
    wkt_d = nc.dram_tensor("wkt", [C, VD], f16, kind="ExternalInput").ap()
    wvt_d = nc.dram_tensor("wvt", [C, VD], f16, kind="ExternalInput").ap()
    wot_d = nc.dram_tensor("wot", [VD, C], f16, kind="ExternalInput").ap()
    bq_d = nc.dram_tensor("bq2", [P, QT], f32, kind="ExternalInput").ap()
    bk_d = nc.dram_tensor("bk2", [P, QT], f32, kind="ExternalInput").ap()
    bvbm_d = nc.dram_tensor("bvbm", [P, MC, VD], f16, kind="ExternalInput").ap()
    gqi_d = nc.dram_tensor("gqi", [P, P], f16, kind="ExternalInput").ap()
    gki_d = nc.dram_tensor("gki", [P, P], f16, kind="ExternalInput").ap()
    ind2_d = nc.dram_tensor("ind2", [P, 2], f16, kind="ExternalInput").ap()
    mask16_d = nc.dram_tensor("mask16", [P, MC], f16, kind="ExternalInput").ap()
    mask32_d = nc.dram_tensor("mask32", [P, MC], f32, kind="ExternalInput").ap()
    y_d = nc.dram_tensor("y", [N, C], f16, kind="ExternalOutput").ap()
    if dbg:
        dbg_ktn = nc.dram_tensor("dbg_ktn", [QT, P, MC * P], f16, kind="ExternalOutput").ap()
        dbg_vt = nc.dram_tensor("dbg_vt", [MC, P, VD], f16, kind="ExternalOutput").ap()
        dbg_qtn = nc.dram_tensor("dbg_qtn", [NT, QT, P, 512], f16, kind="ExternalOutput").ap()
        dbg_rec = nc.dram_tensor("dbg_rec", [NT, P, 512], f16, kind="ExternalOutput").ap()
        dbg_pt = nc.dram_tensor("dbg_pt", [MC, 2, P, 1024], f16, kind="ExternalOutput").ap()
        dbg_outtn = nc.dram_tensor("dbg_outtn", [QT, P, N], f16, kind="ExternalOutput").ap()

    with tile.TileContext(nc) as tc:
        with (
            tc.tile_pool(name="consts", bufs=1) as consts,
            tc.tile_pool(name="xpool", bufs=1) as xpool,
            tc.tile_pool(name="kv", bufs=1) as kvp,
            tc.tile_pool(name="work", bufs=2) as work,
            tc.tile_pool(name="ptp", bufs=5) as ptp,
            tc.tile_pool(name="outp", bufs=1) as outp,
            # PSUM: "big" = [128,1024] 2-bank tiles (scores pairs, KV proj,
            # out-proj), bufs=2 -> 4 banks.  "acc" = [128,512] 1-bank tiles
            # (pv01, pv23, den persist per nt), bufs=3 -> 3 banks.
            # "mi" = [128,512] 1-bank (Qproj halves / ss / bc), bufs=1.
            tc.tile_pool(name="big", bufs=2, space="PSUM") as bigp,
            tc.tile_pool(name="acc", bufs=3, space="PSUM") as accp,
            tc.tile_pool(name="mi", bufs=1, space="PSUM") as mip,
        ):
            # ---- constants / weights ----
            wq_sb = consts.tile([P, CC, VD], f16)
            nc.sync.dma_start(wq_sb[:], wqt_d.rearrange("(o p) m -> p o m", p=P))
            wk_sb = consts.tile([P, CC, VD], f16)
            nc.sync.dma_start(wk_sb[:], wkt_d.rearrange("(o p) m -> p o m", p=P))
            wv_sb = consts.tile([P, CC, VD], f16)
            nc.sync.dma_start(wv_sb[:], wvt_d.rearrange("(o p) m -> p o m", p=P))
            wo_sb = consts.tile([P, QT, C], f16)
            nc.sync.dma_start(wo_sb[:], wot_d.rearrange("(o p) m -> p o m", p=P))
            bq_sb = consts.tile([P, QT], f32)
            nc.sync.dma_start(bq_sb[:], bq_d[:])
            bk_sb = consts.tile([P, QT], f32)
            nc.sync.dma_start(bk_sb[:], bk_d[:])
            bvbm_sb = consts.tile([P, MC, VD], f16)
            nc.sync.dma_start(bvbm_sb[:], bvbm_d[:])
            gqi_sb = consts.tile([P, P], f16)
            nc.sync.dma_start(gqi_sb[:], gqi_d[:])
            gki_sb = consts.tile([P, P], f16)
            nc.sync.dma_start(gki_sb[:], gki_d[:])
            ind2_sb = consts.tile([P, 2], f16)
            nc.sync.dma_start(ind2_sb[:], ind2_d[:])
            m16_sb = consts.tile([P, MC], f16)
            nc.sync.dma_start(m16_sb[:], mask16_d[:])
            m32_sb = consts.tile([P, MC], f32)
            nc.sync.dma_start(m32_sb[:], mask32_d[:])
            eps_sb = consts.tile([P, 1], f32)
            nc.vector.memset(eps_sb[:], EPS)
            ones64_sb = consts.tile([P, 64], f16)
            nc.vector.memset(ones64_sb[:], 1.0)

            # full x resident in SBUF (fp16, 64KB/partition)
            ctx_sb = xpool.tile([P, CC, MP], f16)
            nc.sync.dma_start(ctx_sb[:], ctxt_d.rearrange("(o p) m -> p o m", p=P))
            xt_sb = xpool.tile([P, CC, N], f16)
            nc.sync.dma_start(xt_sb[:], xt_d.rearrange("(o p) n -> p o n", p=P))

            # ================= KV phase =================
            # K projection: out [kdim, m] (2 tiles of 128 kdims)
            ktn = [kvp.tile([P, MP], f16, name=f"ktn{t}") for t in range(QT)]
            kraw = [kvp.tile([P, MP], f16, name=f"kraw{t}") for t in range(QT)]
            for t in range(QT):
                ps_k = bigp.tile([P, 1024], f32, tag="big")
                for cc in range(CC):
                    for ms in range(0, MP, 512):
                        me = min(ms + 512, MP)
                        nc.tensor.matmul(
                            ps_k[:, ms:me],
                            wk_sb[:, cc, t * P : (t + 1) * P],
                            ctx_sb[:, cc, ms:me],
                            start=(cc == 0),
                            stop=(cc == CC - 1),
                        )
                nc.vector.tensor_scalar_add(
                    kraw[t][:], ps_k[:, :MP], bk_sb[:, t : t + 1]
                )
                sq = work.tile([P, MP], f16, tag="ksq", name="ksq")
                nc.vector.tensor_mul(sq[:], kraw[t][:], kraw[t][:])
                rsl = slice(32 * t, 32 * t + 2)
                ps_ss = bigp.tile([P, 1024], f32, tag="big", name=f"kss{t}")
                for ms in range(0, MP, 512):
                    me = min(ms + 512, MP)
                    nc.tensor.matmul(
                        ps_ss[rsl, ms:me],
                        ind2_sb[:],
                        sq[:, ms:me],
                        start=True,
                        stop=True,
                    )
                # rsqrt(mean_sq + eps) = Exp(-0.5 * Ln(ss/D + eps)); Ln and
                # Exp share one ACT table set so no table switches ever.
                srt = work.tile([34, MP], f32, tag="ksrt", name="ksrt", bufs=1)
                nc.scalar.activation(
                    srt[rsl, :], ps_ss[rsl, :MP], AF.Ln, scale=1.0 / D,
                    bias=eps_sb[rsl, :],
                )
                rstd16 = work.tile([34, MP], f16, tag="krstd16", name="krstd16")
                nc.scalar.activation(rstd16[rsl, :], srt[rsl, :], AF.Exp, scale=-0.5)
                ps_bc = bigp.tile([P, 1024], f32, tag="big", name=f"kbc{t}")
                for ms in range(0, MP, 512):
                    me = min(ms + 512, MP)
                    nc.tensor.matmul(
                        ps_bc[:, ms:me],
                        gki_sb[rsl, :],
                        rstd16[rsl, ms:me],
                        start=True,
                        stop=True,
                    )
                nc.vector.tensor_mul(ktn[t][:], kraw[t][:], ps_bc[:, :MP])
                if dbg:
                    nc.sync.dma_start(dbg_ktn[t], ktn[t][:])

            # V projection directly in [m, vdim] layout + bias + mask
            vt = []
            for mc in range(MC):
                ps_v = mip.tile([P, 512], f32, tag="mi", name=f"v{mc}")
                for cc in range(CC):
                    nc.tensor.matmul(
                        ps_v[:, 0:VD],
                        ctx_sb[:, cc, mc * P : (mc + 1) * P],
                        wv_sb[:, cc, :],
                        start=(cc == 0),
                        stop=(cc == CC - 1),
                    )
                vtile = kvp.tile([P, VD], f16, name=f"vt{mc}")
                # v = vproj * maskcol + (bv * maskcol)
                nc.vector.scalar_tensor_tensor(
                    out=vtile[:],
                    in0=ps_v[:, 0:VD],
                    scalar=m32_sb[:, mc : mc + 1],
                    in1=bvbm_sb[:, mc, :],
                    op0=MUL,
                    op1=mybir.AluOpType.add,
                )
                vt.append(vtile)
                if dbg:
                    nc.sync.dma_start(dbg_vt[mc], vtile[:])

            # ================= main pipelined loop =================
            # state carried between iterations
            qtn_tiles = [None, None]  # current nt's normalized q (2 tiles)
            prev = None  # (qtn tiles, ) of previous nt awaiting attention

            def issue_qproj_half(nt, t, dst_raw):
                """Q projection for qdim tile t of block nt -> raw16."""
                nsl = slice(nt * 512, (nt + 1) * 512)
                ps_q = mip.tile([P, 512], f32, tag="mi", name=f"q{nt}_{t}")
                for cc in range(CC):
                    nc.tensor.matmul(
                        ps_q[:],
                        wq_sb[:, cc, t * P : (t + 1) * P],
                        xt_sb[:, cc, nsl],
                        start=(cc == 0),
                        stop=(cc == CC - 1),
                    )
                nc.vector.tensor_scalar_add(dst_raw[:], ps_q[:], bq_sb[:, t : t + 1])

            # Attention for block `nt` uses tiles from `state`:
            #   qtn (2 tiles [128,512] f16), produces outtn via pv/den.
            outtn = [
                outp.tile([P, N], f16, name=f"outtn{t}") for t in range(QT)
            ]

            for step in range(NT + 1):
                do_q = step < NT
                do_attn = step > 0
                ant = step - 1  # attention block index

                # ---- rms/proj state for this step's Q block ----
                if do_q:
                    raw16 = [
                        work.tile([P, 512], f16, tag=f"qraw{t}", name=f"qraw{t}")
                        for t in range(QT)
                    ]
                    sq16 = [
                        work.tile([P, 512], f16, tag=f"qsq{t}", name=f"qsq{t}")
                        for t in range(QT)
                    ]
                    qtn_tiles = [
                        work.tile([P, 512], f16, tag=f"qtn{t}", name=f"qtn{t}")
                        for t in range(QT)
                    ]

                if do_attn:
                    aqtn = prev
                    ps_pv = [
                        accp.tile([P, 512], f32, tag="acc", name=f"pv{pr}")
                        for pr in range(2)
                    ]
                    ps_den = accp.tile([P, 512], f32, tag="acc", name="den")
                    pt_tiles = []

                # interleave: for each mc, issue scores(mc), exp(mc),
                # then a slice of Q-proj work, then pv/den(mc-1).
                n_mc = MC if do_attn else 0

                def qwork_slice(i):
                    """Issue the i-th slice of this step's Q-proj/rms work."""
                    if not do_q:
                        return
                    if i == 0:
                        issue_qproj_half(step, 0, raw16[0])
                        nc.vector.tensor_mul(sq16[0][:], raw16[0][:], raw16[0][:])
                    elif i == 1:
                        issue_qproj_half(step, 1, raw16[1])
                        nc.vector.tensor_mul(sq16[1][:], raw16[1][:], raw16[1][:])
                    elif i == 2:
                        # ss col-tiled pair: rows 0:2 (tile0) and 32:34 (tile1)
                        ps_ss = mip.tile([P, 512], f32, tag="mi", name="qss")
                        nc.tensor.matmul(
                            ps_ss[0:2, :], ind2_sb[:], sq16[0][:],
                            start=True, stop=True,
                        )
                        nc.tensor.matmul(
                            ps_ss[32:34, :], ind2_sb[:], sq16[1][:],
                            start=True, stop=True,
                        )
                        srt = work.tile([34, 512], f32, tag="qsrt", name="qsrt")
                        nc.scalar.activation(
                            srt[:], ps_ss[0:34, :], AF.Ln, scale=1.0 / D,
                            bias=eps_sb[0:34, :],
                        )
                        r16 = work.tile([34, 512], f16, tag="qr16", name="qr16")
                        nc.scalar.activation(r16[:], srt[:], AF.Exp, scale=-0.5)
                        self_r16[0] = r16
                    elif i in (3, 4):
                        t = i - 3
                        r16 = self_r16[0]
                        ps_bc = mip.tile([P, 512], f32, tag="mi", name=f"qbc{t}")
                        nc.tensor.matmul(
                            ps_bc[:],
                            gqi_sb[32 * t : 32 * t + 2, :],
                            r16[32 * t : 32 * t + 2, :],
                            start=True,
                            stop=True,
                        )
                        nc.vector.tensor_mul(qtn_tiles[t][:], raw16[t][:], ps_bc[:])
                        if dbg:
                            nc.sync.dma_start(dbg_qtn[step, t], qtn_tiles[t][:])

                self_r16 = [None]

                if not do_attn:
                    for i in range(5):
                        qwork_slice(i)
                else:
                    ansl = slice(ant * 512, (ant + 1) * 512)
                    qi = 0
                    for mc in range(n_mc + 1):
                        if mc < n_mc:
                            # scores for both head pairs, row-tiled (K=64)
                            pt_pair = []
                            for pr in range(2):
                                ps_s = bigp.tile(
                                    [P, 1024], f32, tag="big", name=f"s{mc}_{pr}"
                                )
                                kt = ktn[pr]
                                qt = aqtn[pr]
                                msl = slice(mc * P, (mc + 1) * P)
                                nc.tensor.matmul(
                                    ps_s[:, 0:512], kt[0:64, msl], qt[0:64, :],
                                    start=True, stop=True,
                                )
                                nc.tensor.matmul(
                                    ps_s[:, 512:1024], kt[64:128, msl], qt[64:128, :],
                                    start=True, stop=True,
                                )
                                pt = ptp.tile([P, 1024], f16, tag="pt")
                                nc.scalar.activation(pt[:], ps_s[:], AF.Exp)
                                if dbg and ant == 0:
                                    nc.sync.dma_start(dbg_pt[mc, pr], pt[:])
                                pt_pair.append(pt)
                            pt_tiles.append(pt_pair)
                        # a slice of Q work between scores and pv
                        if qi < 5:
                            qwork_slice(qi)
                            qi += 1
                        # pv/den for previous mc (lag 1)
                        pmc = mc - 1
                        if 0 <= pmc:
                            pt_pair = pt_tiles[pmc]
                            for pr in range(2):
                                pt = pt_pair[pr]
                                for hh in range(2):
                                    h = 2 * pr + hh
                                    nc.tensor.matmul(
                                        ps_pv[pr][64 * hh : 64 * hh + 64, :],
                                        vt[pmc][:, 64 * h : 64 * h + 64],
                                        pt[:, 512 * hh : 512 * hh + 512],
                                        start=(pmc == 0),
                                        stop=(pmc == MC - 1),
                                    )
                            for pr in range(2):
                                pt = pt_pair[pr]
                                for hh in range(2):
                                    h = 2 * pr + hh
                                    nc.tensor.matmul(
                                        ps_den[32 * h : 32 * h + 1, :],
                                        m16_sb[:, pmc : pmc + 1],
                                        pt[:, 512 * hh : 512 * hh + 512],
                                        start=(pmc == 0),
                                        stop=(pmc == MC - 1),
                                        tile_position=(0, 32 * h),
                                    )
                    while qi < 5:
                        qwork_slice(qi)
                        qi += 1

                    # ---- normalize -> outtn ----
                    # ps_den holds den/256 (mask stationary is 1/256), so the
                    # fp16 reciprocal 256/den stays in normal fp16 range; the
                    # stt scalar 1/256 compensates exactly.
                    rd32 = work.tile([P, 512], f32, tag="rd32", name="rd32")
                    nc.vector.reciprocal_approx_fast(
                        out=rd32[0:97, :], in_=ps_den[0:97, :]
                    )
                    rd16 = work.tile([P, 512], f16, tag="rd16", name="rd16")
                    nc.vector.tensor_copy(rd16[0:97, :], rd32[0:97, :])
                    if dbg:
                        nc.sync.dma_start(dbg_rec[ant], rd16[:])
                    for pr in range(2):
                        # broadcast each head's recip row across 64 partitions
                        # with a K=1 matmul (ones column stationary)
                        ps_bcn = mip.tile([P, 512], f32, tag="mi", name=f"bcn{pr}")
                        for hh in range(2):
                            h = 2 * pr + hh
                            nc.tensor.matmul(
                                ps_bcn[64 * hh : 64 * hh + 64, :],
                                ones64_sb[32 * h : 32 * h + 1, :],
                                rd16[32 * h : 32 * h + 1, :],
                                start=True,
                                stop=True,
                                tile_position=(32 * h, 64 * hh),
                            )
                        bcn_sb = work.tile(
                            [P, 512], f16, tag=f"bcn{pr}", name=f"bcn{pr}"
                        )
                        nc.vector.tensor_copy(bcn_sb[:], ps_bcn[:])
                        nc.vector.scalar_tensor_tensor(
                            out=outtn[pr][:, ansl],
                            in0=ps_pv[pr][:],
                            scalar=1.0 / 256.0,
                            in1=bcn_sb[:],
                            op0=MUL,
                            op1=MUL,
                        )

                prev = qtn_tiles

            if dbg:
                for t in range(QT):
                    nc.sync.dma_start(dbg_outtn[t], outtn[t][:])

            # ================= output projection =================
            for tcn in range(N // P):
                ps_y = bigp.tile([P, 1024], f32, tag="big", name="ps_y")
                tsl = slice(tcn * P, (tcn + 1) * P)
                for half in range(2):
                    ysl = slice(half * 512, (half + 1) * 512)
                    for t in range(QT):
                        nc.tensor.matmul(
                            ps_y[:, ysl],
                            outtn[t][:, tsl],
                            wo_sb[:, t, ysl],
                            start=(t == 0),
                            stop=(t == QT - 1),
                        )
                y_sb = work.tile([P, C], f16, tag="ysb", name="ysb")
                if tcn % 2 == 0:
                    nc.scalar.activation(y_sb[:, 0:512], ps_y[:, 0:512], AF.Copy)
                    nc.vector.tensor_copy(y_sb[:, 512:1024], ps_y[:, 512:1024])
                else:
                    nc.vector.tensor_copy(y_sb[:, 0:512], ps_y[:, 0:512])
                    nc.scalar.activation(y_sb[:, 512:1024], ps_y[:, 512:1024], AF.Copy)
                nc.sync.dma_start(y_d[tsl, :], y_sb[:])

    nc.compile()
    _CACHE[key] = nc
    return nc


def _prep(x, context, context_mask, Wq, bq, Wkv, bkv, gq, gk, Wo, bo):
    """Host-side: compaction, transposes, per-core weight slices."""
    f16 = np.float16
    f32 = np.float32
    mask = np.asarray(context_mask)
    idxs = [np.nonzero(mask[b])[0] for b in range(B)]
    mv = [len(ix) for ix in idxs]
    MC = max(1, (max(mv) + P - 1) // P)
    MP = MC * P

    # compacted, padded, transposed context per batch (fp16)
    ctxt = []
    for b in range(B):
        cc = np.zeros((MP, C), dtype=f32)
        cc[: mv[b]] = np.asarray(context[b], dtype=f32)[idxs[b]]
        ctxt.append(np.ascontiguousarray(cc.T, dtype=f16))

    # mask columns [128, MC] per batch
    m32 = []
    for b in range(B):
        m = np.zeros((MP,), dtype=f32)
        m[: mv[b]] = 1.0
        m32.append(np.ascontiguousarray(m.reshape(MC, P).T))

    xt = [
        np.ascontiguousarray(np.asarray(x[b], dtype=f32).T, dtype=f16)
        for b in range(B)
    ]

    ind2 = np.zeros((P, 2), dtype=f16)
    ind2[0:64, 0] = 1.0
    ind2[64:128, 1] = 1.0

    Wq = np.asarray(Wq, dtype=f32)
    Wkv = np.asarray(Wkv, dtype=f32)
    Wo = np.asarray(Wo, dtype=f32)
    bq = np.asarray(bq, dtype=f32)
    bkv = np.asarray(bkv, dtype=f32)
    gq = np.asarray(gq, dtype=f32)
    gk = np.asarray(gk, dtype=f32)

    in_maps = []
    for c in range(8):
        bi, hg = c // 4, c % 4
        hs = slice(VD * hg, VD * (hg + 1))  # 256 dims for 4 heads
        heads = [hg * HC + j for j in range(HC)]

        gqi = np.zeros((P, P), dtype=f16)
        gki = np.zeros((P, P), dtype=f16)
        for t in range(QT):
            for j in range(2):
                h = heads[2 * t + j]
                gqi[32 * t + j, 64 * j : 64 * j + 64] = (
                    gq[h] * (1.0 / np.sqrt(D))
                ).astype(f16)
                gki[32 * t + j, 64 * j : 64 * j + 64] = gk[h].astype(f16)

        bv = bkv[C + VD * hg : C + VD * (hg + 1)]
        bvbm = np.zeros((P, MC, VD), dtype=f16)
        for mc in range(MC):
            bvbm[:, mc, :] = (
                m32[bi][:, mc : mc + 1] * bv[None, :]
            ).astype(f16)

        in_maps.append(
            {
                "xt": xt[bi],
                "ctxt": ctxt[bi],
                "wqt": np.ascontiguousarray(Wq[hs].T, dtype=f16),
                "wkt": np.ascontiguousarray(Wkv[hs].T, dtype=f16),
                "wvt": np.ascontiguousarray(
                    Wkv[C + VD * hg : C + VD * (hg + 1)].T, dtype=f16
                ),
                "wot": np.ascontiguousarray(Wo[:, hs].T, dtype=f16),
                "bq2": np.ascontiguousarray(
                    bq[hs].reshape(QT, P).T, dtype=f32
                ),
                "bk2": np.ascontiguousarray(
                    bkv[hs].reshape(QT, P).T, dtype=f32
                ),
                "bvbm": bvbm,
                "gqi": gqi,
                "gki": gki,
                "ind2": ind2,
                "mask16": (m32[bi] / 256.0).astype(f16),
                "mask32": m32[bi],
            }
        )
    return in_maps, MC


def _run(in_maps, MC, **spmd_kwargs):
    from concourse import bass_utils

    nc = _build(MC)
    return bass_utils.run_bass_kernel_spmd(
        nc, in_maps, core_ids=list(range(8)), **spmd_kwargs
    )


def kernel(x, context, context_mask, Wq, bq, Wkv, bkv, gq, gk, Wo, bo):
    in_maps, MC = _prep(
        x, context, context_mask, Wq, bq, Wkv, bkv, gq, gk, Wo, bo
    )
    res = _run(in_maps, MC)
    y = np.zeros((B, N, C), dtype=np.float64)
    for c in range(8):
        y[c // 4] += res.results[c]["y"].astype(np.float64)
    y += np.asarray(bo, dtype=np.float64)[None, None, :]
    return y.astype(np.float32)
